# revision 11
# baseline (speedup 1.0000x reference)
"""FFM (field-aware factorization machine) forward pass on 8 Trainium2 cores.

Math (per sample b):
    linear[b] = X[b,:] @ w1 + b0
    C[i,j]    = sum_k v[i, field[j], k] * v[j, field[i], k]   (pair coefficients)
    inter[b]  = sum_{i<j} C[i,j] X[b,i] X[b,j]
    out[b]    = sigmoid(linear[b] + inter[b])

Strategy (v2 -- fp8 DoubleRow + split epilogue + issue-cheap DMA):
  * Precompute Cm = strict-upper(C) on host, fold w1^T into the (structurally
    zero) row 511, scale by 2^SC into fp8e4m3 range; X also goes to fp8 for
    the matmul (plus a bf16 natural-layout copy for the epilogue).  The
    sigmoid's free scale argument multiplies by 2^-SC at the end.
  * Y = X@Cm via fp8 DoubleRow matmuls: contraction 512 = 2 chunks of 256
    (2 packed k-tiles each).  Chunk A = k-tiles {0,3} (full width, carries
    the w1 row), chunk B = k-tiles {1,2} (strict-upper trim: cols 128..511).
    2 matmuls/tile instead of 4, at ~2x fp8 element rate.
  * Split epilogue: rowsum(Y*X) with the PSUM drain shared between engines
    by TILE PARITY (ScalarE+VectorE may not touch the same psum bank
    concurrently -- hw collision abort).  Even tiles: VectorE drains psum
    directly with one fp32 STT.  Odd tiles: ScalarE (closer to PSUM) copies
    the whole bank to bf16 SBUF; VectorE then multiplies it against X in
    2x-mode bf16.  One shared accumulator array, no merge step (reading a
    DVE accum_out from the NEXT DVE op races the accumulator flush; all acc
    reads here are cross-engine behind semaphores).  ScalarE also does the
    batched sigmoids.
  * All DRAM layouts are per-partition contiguous so each dma_start emits
    128 large descriptors (the baseline's strided layouts cost ~1.1us of
    sequencer DIRECT2D per dma_start).  Issue lanes: sync = C + X^T groups
    + outputs, scalar = bias + first natural-X groups (before its copy loop
    starts), gpsimd (SWDGE) = remaining natural-X groups.
  * A few dummy DoubleRow matmuls at stream start warm the PE HAM clock
    gate while the first DMA groups land.

Raw bass (no TileContext: this container's walrus rejects Tile's multi-wait
encodings and the TENSOR_TENSOR_REDUCE direct-ISA opcode).
"""

import contextlib

import numpy as np
import ml_dtypes

P = 128          # partitions / tile rows
F = 512          # features
NCORES = 8
B = 32768
BSH = B // NCORES   # 4096 rows per core
NT = BSH // P       # 32 batch tiles per core
NPAIR = 3           # psum bank-pair rotation depth (2 banks each)
NWARM = 6           # dummy warm-up matmuls bridging the first DMA arrivals
KM = ((0, 3), (1, 2))   # k-tile pairing for the two DoubleRow chunks
CB_J0 = 128             # chunk B column base (strict-upper trim)

BF16 = ml_dtypes.bfloat16
FP8 = ml_dtypes.float8_e4m3


def _groups(singles, pairs_until, quad):
    gs = [(t, 1) for t in range(singles)]
    t = singles
    while t < pairs_until:
        gs.append((t, 2))
        t += 2
    while t < NT:
        n = min(quad, NT - t)
        gs.append((t, n))
        t += n
    return gs


XT_G = _groups(2, 6, 8)     # [(0,1),(1,1),(2,2),(4,2),(6,8),(14,8),(22,8),(30,2)]
XN_G = _groups(2, 6, 8)
# NOTE: gpsimd (SWDGE) dma_start hangs multi-core runs in this container --
# all DMA goes through the two HWDGE lanes (sync, scalar).


def _build_bass(sc_pow):
    import concourse.bass as bass
    from concourse import mybir

    nc = bass.Bass()

    xn_d = nc.declare_dram_parameter("xn", [P, NT, F], mybir.dt.bfloat16, isOutput=False)[:]
    xt_d = nc.declare_dram_parameter("xt", [P, NT, 2, 2, P], mybir.dt.float8e4, isOutput=False)[:]
    ca_d = nc.declare_dram_parameter("ca", [P, 2, F], mybir.dt.float8e4, isOutput=False)[:]
    cb_d = nc.declare_dram_parameter("cb", [P, 2, F - CB_J0], mybir.dt.float8e4, isOutput=False)[:]
    bias = nc.declare_dram_parameter("bias", [1], mybir.dt.float32, isOutput=False)[:]
    y = nc.declare_dram_parameter("y", [P, NT], mybir.dt.float32, isOutput=True)[:]

    xt_of = {}
    for gi, (t0, n) in enumerate(XT_G):
        for t in range(t0, t0 + n):
            xt_of[t] = gi
    xn_of = {}
    for gi, (t0, n) in enumerate(XN_G):
        for t in range(t0, t0 + n):
            xn_of[t] = gi

    DR = mybir.MatmulPerfMode.DoubleRow

    with contextlib.ExitStack() as st:
        ec = st.enter_context
        ca_sb = ec(nc.sbuf_tensor([P, 2, F], mybir.dt.float8e4))
        cb_sb = ec(nc.sbuf_tensor([P, 2, F - CB_J0], mybir.dt.float8e4))
        xbuf = ec(nc.sbuf_tensor([P, NT, F], mybir.dt.bfloat16))
        xtbuf = ec(nc.sbuf_tensor([P, NT, 2, 2, P], mybir.dt.float8e4))
        ycopy = ec(nc.sbuf_tensor([P, 2, F], mybir.dt.bfloat16))
        dump = ec(nc.sbuf_tensor([P, F], mybir.dt.bfloat16))
        acc1 = ec(nc.sbuf_tensor([P, NT], mybir.dt.float32))
        out_sb = ec(nc.sbuf_tensor([P, NT], mybir.dt.float32))
        b_sb = ec(nc.sbuf_tensor([P, 1], mybir.dt.float32))
        pp = [ec(nc.psum_tensor(f"pp{i}", [P, 2, F], mybir.dt.float32)) for i in range(NPAIR)]
        ps_warm = ec(nc.psum_tensor("ps_warm", [P, F], mybir.dt.float32))

        s_ca = ec(nc.semaphore(name="s_ca"))
        s_cb = ec(nc.semaphore(name="s_cb"))
        s_b = ec(nc.semaphore(name="s_b"))
        s_xt = [ec(nc.semaphore(name=f"s_xt{i}")) for i in range(len(XT_G))]
        s_xn = [ec(nc.semaphore(name=f"s_xn{i}")) for i in range(len(XN_G))]
        s_mm = ec(nc.semaphore(name="s_mm"))    # +1 per tile (tensor)
        s_cp = ec(nc.semaphore(name="s_cp"))    # +1 per odd-tile scalar copy
        s_d2 = ec(nc.semaphore(name="s_d2"))    # +1 per pair fully drained by DVE
        s_act = ec(nc.semaphore(name="s_act"))
        s_out = ec(nc.semaphore(name="s_out"))

        block = ec(nc.Block())

        @block.sync
        def _(sync):
            sync.dma_start(out=ca_sb[:], in_=ca_d).then_inc(s_ca, 16)
            sync.dma_start(out=cb_sb[:], in_=cb_d).then_inc(s_cb, 16)
            # interleave xt / xn groups by first-use time (xn feeds DVE,
            # which lags PE by ~1 tile -- keep both streams fed in lockstep)
            xn_left = list(range(len(XN_G)))
            for gi, (t0, n) in enumerate(XT_G):
                sync.dma_start(
                    out=xtbuf[:, t0 : t0 + n], in_=xt_d[:, t0 : t0 + n]
                ).then_inc(s_xt[gi], 16)
                while xn_left and XN_G[xn_left[0]][0] <= t0:
                    gj = xn_left.pop(0)
                    u0, un = XN_G[gj]
                    sync.dma_start(
                        out=xbuf[:, u0 : u0 + un], in_=xn_d[:, u0 : u0 + un]
                    ).then_inc(s_xn[gj], 16)
            for gj in xn_left:
                u0, un = XN_G[gj]
                sync.dma_start(
                    out=xbuf[:, u0 : u0 + un], in_=xn_d[:, u0 : u0 + un]
                ).then_inc(s_xn[gj], 16)
            # outputs: one store per sigmoid group (4 tiles)
            for yo in range(NT // 4):
                sync.wait_ge(s_act, yo + 1)
                sync.dma_start(
                    out=y[:, 4 * yo : 4 * yo + 4], in_=out_sb[:, 4 * yo : 4 * yo + 4]
                ).then_inc(s_out, 16)
            sync.wait_ge(s_out, 16 * (NT // 4))

        @block.scalar
        def _(scalar):
            scalar.dma_start(out=b_sb[:], in_=bias.to_broadcast([P, 1])).then_inc(s_b, 16)
            scalar.wait_ge(s_b, 16)
            # dummy sigmoid: pull the ~1.3us ACT_TABLE_LOAD off the critical
            # path (hides under the DMA ramp)
            nc.scalar.activation(
                out=out_sb[:, 0:1],
                in_=b_sb[:],
                func=mybir.ActivationFunctionType.Sigmoid,
                bias=b_sb[:],
                scale=1.0,
            )
            for q in range(NT // 2):        # odd tile of each psum pair
                t = 2 * q + 1
                scalar.wait_ge(s_mm, t + 1)
                if q >= 2:
                    # ycopy 2-slot rotation: slot q%2 last held pair q-2
                    scalar.wait_ge(s_d2, q - 1)
                nc.scalar.activation(
                    out=ycopy[:, q % 2, :],
                    in_=pp[q % NPAIR][:, 1, :],
                    func=mybir.ActivationFunctionType.Copy,
                ).then_inc(s_cp, 1)
                if q % 2 == 1:
                    m = q // 2
                    scalar.wait_ge(s_d2, 2 * m + 2)
                    nc.scalar.activation(
                        out=out_sb[:, 4 * m : 4 * m + 4],
                        in_=acc1[:, 4 * m : 4 * m + 4],
                        func=mybir.ActivationFunctionType.Sigmoid,
                        bias=b_sb[:],
                        scale=float(2.0 ** (-sc_pow)),
                    ).then_inc(s_act, 1)

        @block.tensor
        def _(tensor):
            for _w in range(NWARM):
                nc.tensor.matmul(
                    ps_warm[:],
                    xtbuf[:, 0, 0, :, :],
                    ca_sb[:, :, :],
                    start=True,
                    stop=True,
                    perf_mode=DR,
                    skip_group_check=True,
                )
            tensor.wait_ge(s_ca, 16)
            tensor.wait_ge(s_cb, 16)
            for t in range(NT):
                gi = xt_of[t]
                if t == XT_G[gi][0]:
                    tensor.wait_ge(s_xt[gi], 16)
                q = t // 2
                if t % 2 == 0 and q >= NPAIR:
                    # reuse of psum pair slot: previous occupant pair q-NPAIR
                    tensor.wait_ge(s_cp, q - NPAIR + 1)
                    tensor.wait_ge(s_d2, q - NPAIR + 1)
                pst = pp[q % NPAIR]
                nc.tensor.matmul(
                    pst[:, t % 2, :],
                    xtbuf[:, t, 0, :, :],
                    ca_sb[:, :, :],
                    start=True,
                    stop=False,
                    perf_mode=DR,
                    skip_group_check=True,
                )
                mm = nc.tensor.matmul(
                    pst[:, t % 2, CB_J0:],
                    xtbuf[:, t, 1, :, :],
                    cb_sb[:, :, :],
                    start=False,
                    stop=True,
                    perf_mode=DR,
                    skip_group_check=True,
                )
                mm.then_inc(s_mm, 1)

        @block.vector
        def _(vector):
            for t in range(NT):
                gi = xn_of[t]
                if t == XN_G[gi][0]:
                    vector.wait_ge(s_xn[gi], 16)
                q = t // 2
                # ScalarE+VectorE never touch the same psum bank: DVE reads
                # only bank 0 of each pair, ScalarE only bank 1.
                if t % 2 == 0:
                    vector.wait_ge(s_mm, t + 1)
                    nc.vector.scalar_tensor_tensor(
                        out=dump[:],
                        in0=pp[q % NPAIR][:, 0, :],
                        scalar=0.0,
                        in1=xbuf[:, t, :],
                        op0=mybir.AluOpType.add,
                        op1=mybir.AluOpType.mult,
                        accum_out=acc1[:, t : t + 1],
                    )
                else:
                    vector.wait_ge(s_cp, q + 1)
                    nc.vector.scalar_tensor_tensor(
                        out=dump[:],
                        in0=ycopy[:, q % 2, :],
                        scalar=0.0,
                        in1=xbuf[:, t, :],
                        op0=mybir.AluOpType.add,
                        op1=mybir.AluOpType.mult,
                        accum_out=acc1[:, t : t + 1],
                    ).then_inc(s_d2, 1)

    return nc


def _host_prep(X, w1, b, v, feature2field):
    """Returns (sc_pow, per-core input maps)."""
    X = np.asarray(X, dtype=np.float32)
    w1 = np.asarray(w1, dtype=np.float32)
    b = np.asarray(b, dtype=np.float32)
    v = np.asarray(v, dtype=np.float32)
    f2f = np.asarray(feature2field, dtype=np.int32)

    # Pair-coefficient matrix: C[i,j] = sum_k v[i, f2f[j], k] * v[j, f2f[i], k]
    A = v[:, f2f, :]                      # [n, n, k]
    C = (A * A.transpose(1, 0, 2)).sum(axis=2)
    Cm = np.triu(C, 1)
    # Fold the linear term: row F-1 of strict-upper Cm is all zeros.
    Cm[F - 1, :] = w1[:, 0]

    # fp8 scaling: put max|Cm| around 160 (fp8e4m3 max = 240)
    maxabs = float(np.abs(Cm).max())
    sc_pow = int(np.floor(np.log2(160.0 / max(maxabs, 1e-30))))
    C8 = (Cm * (2.0 ** sc_pow)).astype(FP8)

    # chunk layouts: c?[p, kt, j] = C8[KM[c][kt]*P + p, j0 + j]
    C8r = C8.reshape(4, P, F)
    ca = np.ascontiguousarray(
        np.stack([C8r[KM[0][0]], C8r[KM[0][1]]], axis=1)
    )                                     # [P, 2, F]
    cb = np.ascontiguousarray(
        np.stack([C8r[KM[1][0], :, CB_J0:], C8r[KM[1][1], :, CB_J0:]], axis=1)
    )                                     # [P, 2, F-CB_J0]

    X8 = X.astype(FP8)
    Xb = X.astype(BF16)
    in_maps = []
    for c in range(NCORES):
        X8c = X8[c * BSH : (c + 1) * BSH]             # [4096, 512]
        Xbc = Xb[c * BSH : (c + 1) * BSH]
        # xt[p, t, ch, kt, b] = X8c[t*P + b, KM[ch][kt]*P + p]
        x4 = X8c.reshape(NT, P, 4, P)                 # [t, b, ktile, p]
        xt = np.ascontiguousarray(
            np.stack(
                [
                    np.stack([x4[:, :, KM[0][0]], x4[:, :, KM[0][1]]], axis=0),
                    np.stack([x4[:, :, KM[1][0]], x4[:, :, KM[1][1]]], axis=0),
                ],
                axis=0,
            ).transpose(4, 2, 0, 1, 3)                # [p, t, ch, kt, b]
        )
        # w1-fold: stationary row for feature 511 (= chunk 0, kt 1, p 127) := 1.0
        xt[P - 1, :, 0, 1, :] = FP8(1.0)
        # xn[p, t, f] = Xbc[t*P + p, f]
        xn = np.ascontiguousarray(Xbc.reshape(NT, P, F).transpose(1, 0, 2))
        in_maps.append({"xn": xn, "xt": xt, "ca": ca, "cb": cb, "bias": b})
    return sc_pow, in_maps


def _run(prep, trace=False):
    from concourse.bass_utils import run_bass_kernel_spmd

    sc_pow, in_maps = prep
    nc = _build_bass(sc_pow)
    res = run_bass_kernel_spmd(nc, in_maps, core_ids=list(range(NCORES)), trace=trace)
    out = np.concatenate([r["y"].reshape(P, NT).T.reshape(-1) for r in res.results])
    return out, res


def kernel(X, w1, b, v, feature2field):
    prep = _host_prep(X, w1, b, v, feature2field)
    out, _ = _run(prep, trace=False)
    return out.astype(np.float32)


if __name__ == "__main__":
    pass


# revision 13
# speedup vs baseline: 1.0351x; 1.0351x over previous
"""FFM (field-aware factorization machine) forward pass on 8 Trainium2 cores.

Math (per sample b):
    linear[b] = X[b,:] @ w1 + b0
    C[i,j]    = sum_k v[i, field[j], k] * v[j, field[i], k]   (pair coefficients)
    inter[b]  = sum_{i<j} C[i,j] X[b,i] X[b,j]
    out[b]    = sigmoid(linear[b] + inter[b])

Strategy (v3 -- fp8 DoubleRow matmuls + hybrid direct/eigen epilogue):
  * inter[b] = x^T Cm x with Cm = strict-upper(C).  Host also eigendecomposes
    S = Cm + Cm^T = Q diag(lam) Q^T, giving the equivalent form
    inter = sum_pos z_r^2 - sum_neg z_r^2 with Z = X W, W = Q sqrt(|lam|/2)
    (columns sorted positive-lambda first).  BOTH forms are evaluated, on
    alternating batch tiles, so the PSUM drain splits across two engines
    that may not share a psum bank:
      - even tiles: Y = X@Cm in psum bank 0 of a pair; VectorE drains it
        with one STT rowsum(Y*X) against a bf16 natural-layout X copy.
      - odd tiles: Z = X@W in psum bank 1; ScalarE alone drains it with two
        Square+accumulate activations (positive / negative lambda columns).
        No natural-layout X needed for odd tiles (halves that DMA stream).
  * The linear term X@w1 is computed on host (X is already being cast /
    relaid out there; one matvec is noise) and folded in by VectorE's tiny
    per-4-tile merge STTs, which also apply the fp8 scale correction
    kappa = 2^(sc_c - 2 sc_w) to the eigen tiles.
  * Matmuls: fp8e4m3 DoubleRow, contraction 512 = 2 chunks of 256 (k-tile
    pairs {0,3} / {1,2}).  Even tiles: Cm chunks (N=512 / N=384 by
    strict-upper trim).  Odd tiles: W chunks (dense, N=512 both).  2 MMs
    per tile instead of 4 bf16 ones.
  * Accumulator-read discipline: a DVE op must not read the accum_out of a
    DVE op issued <2 instructions earlier (accumulator flush race, found
    the hard way); cross-engine reads behind semaphores are safe.  Merges
    are scheduled 2+ ops behind the acc writes they read.
  * ScalarE+VectorE may never touch the same psum bank concurrently (HW
    collision abort) -- the even/odd bank split guarantees that.
  * All DRAM layouts are per-partition contiguous (cheap descriptor gen);
    both HWDGE lanes issue: sync = C/W + X^T + even-X + outputs, scalar =
    bias (gpsimd SWDGE dma hangs multi-core runs in this container).  A
    dummy sigmoid right after the bias load pulls the ~1.3us ACT table
    load off the critical path, and sigmoids lag the square stream by one
    group so they never stall the scalar queue.
  * A few dummy DoubleRow matmuls at stream start warm the PE HAM clock
    gate while the first DMA groups land.

Raw bass (no TileContext: this container's walrus rejects Tile's multi-wait
encodings and the TENSOR_TENSOR_REDUCE direct-ISA opcode).
"""

import contextlib

import numpy as np
import ml_dtypes

P = 128          # partitions / tile rows
F = 512          # features
NCORES = 8
B = 32768
BSH = B // NCORES   # 4096 rows per core
NT = BSH // P       # 32 batch tiles per core
NE = NT // 2        # even tiles (direct path) / odd tiles (eigen path)
NPAIR = 3           # psum bank-pair rotation depth (2 banks each)
NWARM = 6           # dummy warm-up matmuls bridging the first DMA arrivals
KM = ((0, 3), (1, 2))   # k-tile pairing for the two DoubleRow chunks
CB_J0 = 128             # Cm chunk B column base (strict-upper trim)

BF16 = ml_dtypes.bfloat16
FP8 = ml_dtypes.float8_e4m3


def _groups(n_total, singles, pairs_until, quad):
    gs = [(t, 1) for t in range(singles)]
    t = singles
    while t < pairs_until:
        gs.append((t, 2))
        t += 2
    while t < n_total:
        n = min(quad, n_total - t)
        gs.append((t, n))
        t += n
    return gs


XT_G = _groups(NT, 2, 6, 8)   # groups over all 32 tiles (fp8 X^T)
XE_G = _groups(NE, 1, 3, 4)   # groups over the 16 even tiles (bf16 natural X)


def _build_bass(sc_c, sc_w, p_pos):
    import concourse.bass as bass
    from concourse import mybir

    nc = bass.Bass()

    xn_d = nc.declare_dram_parameter("xn", [P, NE, F], mybir.dt.bfloat16, isOutput=False)[:]
    xt_d = nc.declare_dram_parameter("xt", [P, NT, 2, 2, P], mybir.dt.float8e4, isOutput=False)[:]
    ca_d = nc.declare_dram_parameter("ca", [P, 2, F], mybir.dt.float8e4, isOutput=False)[:]
    cb_d = nc.declare_dram_parameter("cb", [P, 2, F - CB_J0], mybir.dt.float8e4, isOutput=False)[:]
    wa_d = nc.declare_dram_parameter("wa", [P, 2, F], mybir.dt.float8e4, isOutput=False)[:]
    wb_d = nc.declare_dram_parameter("wb", [P, 2, F], mybir.dt.float8e4, isOutput=False)[:]
    lin_d = nc.declare_dram_parameter("lin", [P, NT], mybir.dt.float32, isOutput=False)[:]
    bias = nc.declare_dram_parameter("bias", [1], mybir.dt.float32, isOutput=False)[:]
    y = nc.declare_dram_parameter("y", [P, NT], mybir.dt.float32, isOutput=True)[:]

    xt_of = {}
    for gi, (t0, n) in enumerate(XT_G):
        for t in range(t0, t0 + n):
            xt_of[t] = gi
    xe_of = {}
    for gi, (e0, n) in enumerate(XE_G):
        for e in range(e0, e0 + n):
            xe_of[e] = gi

    DR = mybir.MatmulPerfMode.DoubleRow
    kappa = float(2.0 ** (sc_c - 2 * sc_w))

    with contextlib.ExitStack() as st:
        ec = st.enter_context
        ca_sb = ec(nc.sbuf_tensor([P, 2, F], mybir.dt.float8e4))
        cb_sb = ec(nc.sbuf_tensor([P, 2, F - CB_J0], mybir.dt.float8e4))
        wa_sb = ec(nc.sbuf_tensor([P, 2, F], mybir.dt.float8e4))
        wb_sb = ec(nc.sbuf_tensor([P, 2, F], mybir.dt.float8e4))
        xbuf = ec(nc.sbuf_tensor([P, NE, F], mybir.dt.bfloat16))
        xtbuf = ec(nc.sbuf_tensor([P, NT, 2, 2, P], mybir.dt.float8e4))
        dump = ec(nc.sbuf_tensor([P, F], mybir.dt.bfloat16))
        dump_s = ec(nc.sbuf_tensor([P, F], mybir.dt.bfloat16))
        acc1 = ec(nc.sbuf_tensor([P, NT], mybir.dt.float32))
        acc_p = ec(nc.sbuf_tensor([P, NE], mybir.dt.float32))
        acc_n = ec(nc.sbuf_tensor([P, NE], mybir.dt.float32))
        tmp2 = ec(nc.sbuf_tensor([P, 2, 2], mybir.dt.float32))   # double-buffered
        accm = ec(nc.sbuf_tensor([P, NT], mybir.dt.float32))
        lin_sb = ec(nc.sbuf_tensor([P, NT], mybir.dt.float32))
        out_sb = ec(nc.sbuf_tensor([P, NT], mybir.dt.float32))
        b_sb = ec(nc.sbuf_tensor([P, 1], mybir.dt.float32))
        pp = [ec(nc.psum_tensor(f"pp{i}", [P, 2, F], mybir.dt.float32)) for i in range(NPAIR)]
        ps_warm = ec(nc.psum_tensor("ps_warm", [P, F], mybir.dt.float32))

        s_ca = ec(nc.semaphore(name="s_ca"))
        s_cb = ec(nc.semaphore(name="s_cb"))
        s_wa = ec(nc.semaphore(name="s_wa"))
        s_wb = ec(nc.semaphore(name="s_wb"))
        s_lin = ec(nc.semaphore(name="s_lin"))
        s_b = ec(nc.semaphore(name="s_b"))
        s_xt = [ec(nc.semaphore(name=f"s_xt{i}")) for i in range(len(XT_G))]
        s_xe = [ec(nc.semaphore(name=f"s_xe{i}")) for i in range(len(XE_G))]
        s_mm = ec(nc.semaphore(name="s_mm"))    # +1 per tile (tensor)
        s_sq = ec(nc.semaphore(name="s_sq"))    # +1 per odd tile (scalar squares)
        s_d1 = ec(nc.semaphore(name="s_d1"))    # +1 per even tile (DVE drain)
        s_mg = ec(nc.semaphore(name="s_mg"))    # +1 per merged 4-tile group (DVE)
        s_act = ec(nc.semaphore(name="s_act"))  # +1 per sigmoid group (scalar)
        s_out = ec(nc.semaphore(name="s_out"))

        block = ec(nc.Block())

        @block.sync
        def _(sync):
            sync.dma_start(out=ca_sb[:], in_=ca_d).then_inc(s_ca, 16)
            sync.dma_start(out=cb_sb[:], in_=cb_d).then_inc(s_cb, 16)
            sync.dma_start(out=xtbuf[:, 0:1], in_=xt_d[:, 0:1]).then_inc(s_xt[0], 16)
            sync.dma_start(out=wa_sb[:], in_=wa_d).then_inc(s_wa, 16)
            sync.dma_start(out=wb_sb[:], in_=wb_d).then_inc(s_wb, 16)
            sync.dma_start(out=lin_sb[:], in_=lin_d).then_inc(s_lin, 16)
            # interleave remaining xt groups with even-X groups by first use
            xe_left = list(range(len(XE_G)))
            for gi, (t0, n) in enumerate(XT_G):
                if gi > 0:
                    sync.dma_start(
                        out=xtbuf[:, t0 : t0 + n], in_=xt_d[:, t0 : t0 + n]
                    ).then_inc(s_xt[gi], 16)
                while xe_left and 2 * XE_G[xe_left[0]][0] <= t0:
                    gj = xe_left.pop(0)
                    u0, un = XE_G[gj]
                    sync.dma_start(
                        out=xbuf[:, u0 : u0 + un], in_=xn_d[:, u0 : u0 + un]
                    ).then_inc(s_xe[gj], 16)
            for gj in xe_left:
                u0, un = XE_G[gj]
                sync.dma_start(
                    out=xbuf[:, u0 : u0 + un], in_=xn_d[:, u0 : u0 + un]
                ).then_inc(s_xe[gj], 16)
            # outputs: one store per sigmoid group (4 tiles)
            for yo in range(NT // 4):
                sync.wait_ge(s_act, yo + 1)
                sync.dma_start(
                    out=y[:, 4 * yo : 4 * yo + 4], in_=out_sb[:, 4 * yo : 4 * yo + 4]
                ).then_inc(s_out, 16)
            sync.wait_ge(s_out, 16 * (NT // 4))

        @block.scalar
        def _(scalar):
            scalar.dma_start(out=b_sb[:], in_=bias.to_broadcast([P, 1])).then_inc(s_b, 16)
            scalar.wait_ge(s_b, 16)
            # dummy sigmoid: pull the ~1.3us ACT_TABLE_LOAD (set contains
            # both sigmoid and square) off the critical path
            nc.scalar.activation(
                out=out_sb[:, 0:1],
                in_=b_sb[:],
                func=mybir.ActivationFunctionType.Sigmoid,
                bias=b_sb[:],
                scale=1.0,
            )
            for q in range(NE):             # odd tile 2q+1: eigen squares
                t = 2 * q + 1
                scalar.wait_ge(s_mm, t + 1)
                nc.scalar.activation(
                    out=dump_s[:, 0:p_pos],
                    in_=pp[q % NPAIR][:, 1, 0:p_pos],
                    func=mybir.ActivationFunctionType.Square,
                    accum_out=acc_p[:, q : q + 1],
                )
                nc.scalar.activation(
                    out=dump_s[:, p_pos:],
                    in_=pp[q % NPAIR][:, 1, p_pos:],
                    func=mybir.ActivationFunctionType.Square,
                    accum_out=acc_n[:, q : q + 1],
                ).then_inc(s_sq, 1)
                # sigmoids lag the square stream so the wait on s_mg
                # never stalls upcoming squares
                if q % 2 == 1 and q >= 5:
                    m = (q - 5) // 2
                    scalar.wait_ge(s_mg, m + 1)
                    nc.scalar.activation(
                        out=out_sb[:, 4 * m : 4 * m + 4],
                        in_=accm[:, 4 * m : 4 * m + 4],
                        func=mybir.ActivationFunctionType.Sigmoid,
                        bias=b_sb[:],
                        scale=float(2.0 ** (-sc_c)),
                    ).then_inc(s_act, 1)
            for m in (NT // 4 - 3, NT // 4 - 2, NT // 4 - 1):
                scalar.wait_ge(s_mg, m + 1)
                nc.scalar.activation(
                    out=out_sb[:, 4 * m : 4 * m + 4],
                    in_=accm[:, 4 * m : 4 * m + 4],
                    func=mybir.ActivationFunctionType.Sigmoid,
                    bias=b_sb[:],
                    scale=float(2.0 ** (-sc_c)),
                ).then_inc(s_act, 1)

        @block.tensor
        def _(tensor):
            for _w in range(NWARM):
                nc.tensor.matmul(
                    ps_warm[:],
                    xtbuf[:, 0, 0, :, :],
                    ca_sb[:, :, :],
                    start=True,
                    stop=True,
                    perf_mode=DR,
                    skip_group_check=True,
                )
            tensor.wait_ge(s_ca, 16)
            tensor.wait_ge(s_cb, 16)
            for t in range(NT):
                gi = xt_of[t]
                if t == XT_G[gi][0]:
                    tensor.wait_ge(s_xt[gi], 16)
                if t == 1:
                    tensor.wait_ge(s_wa, 16)
                    tensor.wait_ge(s_wb, 16)
                q = t // 2
                if t % 2 == 0 and q >= NPAIR:
                    # psum pair slot reuse: both consumers of pair q-NPAIR done
                    tensor.wait_ge(s_d1, q - NPAIR + 1)
                    tensor.wait_ge(s_sq, q - NPAIR + 1)
                pst = pp[q % NPAIR]
                if t % 2 == 0:
                    nc.tensor.matmul(
                        pst[:, 0, :],
                        xtbuf[:, t, 0, :, :],
                        ca_sb[:, :, :],
                        start=True,
                        stop=False,
                        perf_mode=DR,
                        skip_group_check=True,
                    )
                    mm = nc.tensor.matmul(
                        pst[:, 0, CB_J0:],
                        xtbuf[:, t, 1, :, :],
                        cb_sb[:, :, :],
                        start=False,
                        stop=True,
                        perf_mode=DR,
                        skip_group_check=True,
                    )
                else:
                    nc.tensor.matmul(
                        pst[:, 1, :],
                        xtbuf[:, t, 0, :, :],
                        wa_sb[:, :, :],
                        start=True,
                        stop=False,
                        perf_mode=DR,
                        skip_group_check=True,
                    )
                    mm = nc.tensor.matmul(
                        pst[:, 1, :],
                        xtbuf[:, t, 1, :, :],
                        wb_sb[:, :, :],
                        start=False,
                        stop=True,
                        perf_mode=DR,
                        skip_group_check=True,
                    )
                mm.then_inc(s_mm, 1)

        @block.vector
        def _(vector):
            vector.wait_ge(s_lin, 16)

            # DVE ops must not read outputs (regular OR accum) of DVE ops
            # issued a few instructions earlier -- the write takes a while to
            # land in SBUF.  Merges are therefore two-phase, >=5 ops apart:
            #   phase1(m): tmp2[m%2] = acc_p - acc_n   (cross-engine reads)
            #   phase2(m): accm[odds] = kappa*tmp2 + lin; accm[evens] = acc1+lin
            def merge_phase1(m):
                vector.wait_ge(s_sq, 2 * m + 2)
                nc.vector.scalar_tensor_tensor(
                    out=tmp2[:, m % 2, :],
                    in0=acc_p[:, 2 * m : 2 * m + 2],
                    scalar=1.0,
                    in1=acc_n[:, 2 * m : 2 * m + 2],
                    op0=mybir.AluOpType.mult,
                    op1=mybir.AluOpType.subtract,
                )

            def merge_phase2(m):
                nc.vector.scalar_tensor_tensor(
                    out=accm[:, 4 * m + 1 : 4 * m + 4 : 2],
                    in0=tmp2[:, m % 2, :],
                    scalar=kappa,
                    in1=lin_sb[:, 4 * m + 1 : 4 * m + 4 : 2],
                    op0=mybir.AluOpType.mult,
                    op1=mybir.AluOpType.add,
                )
                nc.vector.scalar_tensor_tensor(
                    out=accm[:, 4 * m : 4 * m + 4 : 2],
                    in0=acc1[:, 4 * m : 4 * m + 4 : 2],
                    scalar=1.0,
                    in1=lin_sb[:, 4 * m : 4 * m + 4 : 2],
                    op0=mybir.AluOpType.mult,
                    op1=mybir.AluOpType.add,
                ).then_inc(s_mg, 1)

            NM = NT // 4
            for e in range(NE):             # even tile 2e: direct drain
                t = 2 * e
                gi = xe_of[e]
                if e == XE_G[gi][0]:
                    vector.wait_ge(s_xe[gi], 16)
                vector.wait_ge(s_mm, t + 1)
                nc.vector.scalar_tensor_tensor(
                    out=dump[:],
                    in0=pp[(t // 2) % NPAIR][:, 0, :],
                    scalar=0.0,
                    in1=xbuf[:, e, :],
                    op0=mybir.AluOpType.add,
                    op1=mybir.AluOpType.mult,
                    accum_out=acc1[:, t : t + 1],
                ).then_inc(s_d1, 1)
                if e >= 3 and e % 2 == 1:
                    m = (e - 3) // 2
                    merge_phase1(m)
                    if m >= 1:
                        merge_phase2(m - 1)
            merge_phase1(NM - 1)
            merge_phase2(NM - 2)
            merge_phase2(NM - 1)

    return nc


def _host_prep(X, w1, b, v, feature2field):
    """Returns (sc_c, sc_w, p_pos, per-core input maps)."""
    X = np.asarray(X, dtype=np.float32)
    w1 = np.asarray(w1, dtype=np.float32)
    b = np.asarray(b, dtype=np.float32)
    v = np.asarray(v, dtype=np.float32)
    f2f = np.asarray(feature2field, dtype=np.int32)

    # Pair-coefficient matrix: C[i,j] = sum_k v[i, f2f[j], k] * v[j, f2f[i], k]
    A = v[:, f2f, :]                      # [n, n, k]
    C = (A * A.transpose(1, 0, 2)).sum(axis=2)
    Cm = np.triu(C, 1)

    # Eigen form: S = Cm + Cm^T = Q diag(lam) Q^T;  inter = sum lam/2 * z^2
    S = Cm + Cm.T
    lam, Q = np.linalg.eigh(S)
    order = np.argsort(-lam)              # positive lambdas first
    lam = lam[order]
    Q = Q[:, order]
    p_pos = int((lam > 0).sum())
    W = Q * np.sqrt(np.abs(lam) / 2.0)[None, :]     # [F, F]

    def scale_pow(m):
        return int(np.floor(np.log2(160.0 / max(float(m), 1e-30))))

    sc_c = scale_pow(np.abs(Cm).max())
    sc_w = scale_pow(np.abs(W).max())
    C8 = (Cm * (2.0 ** sc_c)).astype(FP8)
    W8 = (W * (2.0 ** sc_w)).astype(FP8)

    C8r = C8.reshape(4, P, F)
    W8r = W8.reshape(4, P, F)
    ca = np.ascontiguousarray(np.stack([C8r[KM[0][0]], C8r[KM[0][1]]], axis=1))
    cb = np.ascontiguousarray(
        np.stack([C8r[KM[1][0], :, CB_J0:], C8r[KM[1][1], :, CB_J0:]], axis=1)
    )
    wa = np.ascontiguousarray(np.stack([W8r[KM[0][0]], W8r[KM[0][1]]], axis=1))
    wb = np.ascontiguousarray(np.stack([W8r[KM[1][0]], W8r[KM[1][1]]], axis=1))

    X8 = X.astype(FP8)
    Xb = X.astype(BF16)
    linv = (X @ w1[:, 0]) * (2.0 ** sc_c)           # [B] fp32, pre-scaled

    in_maps = []
    for c in range(NCORES):
        X8c = X8[c * BSH : (c + 1) * BSH]
        # xt[p, t, ch, kt, b] = X8c[t*P + b, KM[ch][kt]*P + p]  (real X; no
        # folded constant row -- the linear term ships via lin)
        x4 = X8c.reshape(NT, P, 4, P)               # [t, b, ktile, p]
        xt = np.ascontiguousarray(
            np.stack(
                [
                    np.stack([x4[:, :, KM[0][0]], x4[:, :, KM[0][1]]], axis=0),
                    np.stack([x4[:, :, KM[1][0]], x4[:, :, KM[1][1]]], axis=0),
                ],
                axis=0,
            ).transpose(4, 2, 0, 1, 3)              # [p, t, ch, kt, b]
        )
        # natural-layout bf16 X for EVEN tiles only
        Xbc = Xb[c * BSH : (c + 1) * BSH].reshape(NT, P, F)
        xn = np.ascontiguousarray(Xbc[0::2].transpose(1, 0, 2))   # [p, e, f]
        lc = linv[c * BSH : (c + 1) * BSH].reshape(NT, P)
        lin = np.ascontiguousarray(lc.T)                          # [p, t]
        in_maps.append(
            {"xn": xn, "xt": xt, "ca": ca, "cb": cb, "wa": wa, "wb": wb,
             "lin": lin, "bias": b}
        )
    return sc_c, sc_w, p_pos, in_maps


def _run(prep, trace=False):
    from concourse.bass_utils import run_bass_kernel_spmd

    sc_c, sc_w, p_pos, in_maps = prep
    nc = _build_bass(sc_c, sc_w, p_pos)
    res = run_bass_kernel_spmd(nc, in_maps, core_ids=list(range(NCORES)), trace=trace)
    out = np.concatenate([r["y"].reshape(P, NT).T.reshape(-1) for r in res.results])
    return out, res


def kernel(X, w1, b, v, feature2field):
    prep = _host_prep(X, w1, b, v, feature2field)
    out, _ = _run(prep, trace=False)
    return out.astype(np.float32)


if __name__ == "__main__":
    pass


# revision 14
# speedup vs baseline: 1.1830x; 1.1429x over previous
"""FFM (field-aware factorization machine) forward pass on 8 Trainium2 cores.

Math (per sample b):
    linear[b] = X[b,:] @ w1 + b0
    C[i,j]    = sum_k v[i, field[j], k] * v[j, field[i], k]   (pair coefficients)
    inter[b]  = sum_{i<j} C[i,j] X[b,i] X[b,j]
    out[b]    = sigmoid(linear[b] + inter[b])

Strategy (v4 -- fp8 DoubleRow matmuls + hybrid direct/eigen epilogue):
  * inter[b] = x^T Cm x with Cm = strict-upper(C).  Host also eigendecomposes
    S = Cm + Cm^T = Q diag(lam) Q^T, giving the equivalent form
    inter = sum_pos z_r^2 - sum_neg z_r^2 with Z = X W, W = Q sqrt(|lam|/2)
    (columns sorted positive-lambda first).  BOTH forms are evaluated, on a
    12/20 tile split, so the PSUM drain runs on two engines that must never
    share a psum bank (HW collision abort):
      - "direct" tiles (20 of 32): Y = X@Cm; VectorE drains the bank with
        one STT rowsum(Y*X) against a bf16 natural-layout X copy.
      - "eigen" tiles (12 of 32, t%8 in {1,5,7}): Z = X@W; ScalarE alone
        drains the bank with two Square+accumulate activations (positive /
        negative lambda columns).  No natural-layout X for these tiles.
    The ratio balances ScalarE (383ns square + 180ns accumulator-readout,
    x2 per tile) against VectorE (one 690ns STT per tile).
  * The linear term X@w1 is computed on host (X is already being cast /
    relaid out there; one matvec is noise) and folded in by VectorE's tiny
    per-4-tile merge STTs, which also apply the fp8 scale correction
    kappa = 2^(sc_c - 2 sc_w) to the eigen tiles.
  * Matmuls: fp8e4m3 DoubleRow, contraction 512 = 2 chunks of 256 (k-tile
    pairs {0,3} / {1,2}).  Direct tiles: Cm chunks (N=512 / N=384 by
    strict-upper trim).  Eigen tiles: W chunks (dense, N=512 both).  2 MMs
    per tile instead of 4 bf16 ones.  Cm and W chunks ship as ONE packed
    DMA (four sub-views of one SBUF tensor).
  * Accumulator/output-read discipline: a DVE op must not read the
    accum_out OR regular output of a DVE op issued a few instructions
    earlier (write-landing race, found the hard way); cross-engine reads
    behind semaphores are safe.  Merges are two-phase, >=3 ops apart.
  * All DRAM layouts are per-partition contiguous (cheap descriptor gen);
    sync HWDGE lane carries everything except bias (gpsimd SWDGE dma hangs
    multi-core runs in this container).  A dummy sigmoid right after the
    bias load pulls the ~1.3us ACT table load off the critical path;
    sigmoids lag the square stream so they never stall the scalar queue.
  * A few dummy DoubleRow matmuls at stream start warm the PE HAM clock
    gate while the first DMA groups land.

Raw bass (no TileContext: this container's walrus rejects Tile's multi-wait
encodings and the TENSOR_TENSOR_REDUCE direct-ISA opcode).
"""

import contextlib

import numpy as np
import ml_dtypes

P = 128          # partitions / tile rows
F = 512          # features
NCORES = 8
B = 32768
BSH = B // NCORES   # 4096 rows per core
NT = BSH // P       # 32 batch tiles per core
NPSUM = 7           # psum bank rotation depth
NWARM = 6           # dummy warm-up matmuls bridging the first DMA arrivals
KM = ((0, 3), (1, 2))   # k-tile pairing for the two DoubleRow chunks
CB_J0 = 128             # Cm chunk B column base (strict-upper trim)
CB_N = F - CB_J0

# tile type: eigen (ScalarE square path) at t%8 in {1,5,7}, else direct (DVE)
EIG = {1, 5, 7}
IS_E = [t % 8 in EIG for t in range(NT)]
T_E = [t for t in range(NT) if IS_E[t]]      # 12 eigen tiles
T_D = [t for t in range(NT) if not IS_E[t]]  # 20 direct tiles
ND, NEI = len(T_D), len(T_E)
ORD_D = {t: i for i, t in enumerate(T_D)}
ORD_E = {t: i for i, t in enumerate(T_E)}

# packed C/W chunk offsets in the combined rhs tensor [P, 2, CW_N]
OFF_CA, OFF_CB, OFF_WA, OFF_WB = 0, F, F + CB_N, 2 * F + CB_N
CW_N = 3 * F + CB_N

BF16 = ml_dtypes.bfloat16
FP8 = ml_dtypes.float8_e4m3


def _groups(n_total, singles, pairs_until, quad):
    gs = [(t, 1) for t in range(singles)]
    t = singles
    while t < pairs_until:
        gs.append((t, 2))
        t += 2
    while t < n_total:
        n = min(quad, n_total - t)
        gs.append((t, n))
        t += n
    return gs


XT_G = _groups(NT, 2, 6, 8)   # groups over all 32 tiles (fp8 X^T)
XE_G = _groups(ND, 1, 3, 4)   # groups over the 20 direct tiles (bf16 nat X)


def _build_bass(sc_c, sc_w, p_pos):
    import concourse.bass as bass
    from concourse import mybir

    nc = bass.Bass()

    xn_d = nc.declare_dram_parameter("xn", [P, ND, F], mybir.dt.bfloat16, isOutput=False)[:]
    xt_d = nc.declare_dram_parameter("xt", [P, NT, 2, 2, P], mybir.dt.float8e4, isOutput=False)[:]
    cw_d = nc.declare_dram_parameter("cw", [P, 2, CW_N], mybir.dt.float8e4, isOutput=False)[:]
    lin_d = nc.declare_dram_parameter("lin", [P, NT], mybir.dt.float32, isOutput=False)[:]
    bias = nc.declare_dram_parameter("bias", [1], mybir.dt.float32, isOutput=False)[:]
    y = nc.declare_dram_parameter("y", [P, NT], mybir.dt.float32, isOutput=True)[:]

    xt_of = {}
    for gi, (t0, n) in enumerate(XT_G):
        for t in range(t0, t0 + n):
            xt_of[t] = gi
    xe_of = {}
    for gi, (e0, n) in enumerate(XE_G):
        for e in range(e0, e0 + n):
            xe_of[e] = gi

    DR = mybir.MatmulPerfMode.DoubleRow
    kappa = float(2.0 ** (sc_c - 2 * sc_w))

    with contextlib.ExitStack() as st:
        ec = st.enter_context
        cw_sb = ec(nc.sbuf_tensor([P, 2, CW_N], mybir.dt.float8e4))
        xbuf = ec(nc.sbuf_tensor([P, ND, F], mybir.dt.bfloat16))
        xtbuf = ec(nc.sbuf_tensor([P, NT, 2, 2, P], mybir.dt.float8e4))
        dump = ec(nc.sbuf_tensor([P, F], mybir.dt.bfloat16))
        dump_s = ec(nc.sbuf_tensor([P, F], mybir.dt.bfloat16))
        acc1 = ec(nc.sbuf_tensor([P, NT], mybir.dt.float32))
        acc_p = ec(nc.sbuf_tensor([P, NEI], mybir.dt.float32))
        acc_n = ec(nc.sbuf_tensor([P, NEI], mybir.dt.float32))
        tmp2 = ec(nc.sbuf_tensor([P, 2, 2], mybir.dt.float32))   # double-buffered
        accm = ec(nc.sbuf_tensor([P, NT], mybir.dt.float32))
        lin_sb = ec(nc.sbuf_tensor([P, NT], mybir.dt.float32))
        out_sb = ec(nc.sbuf_tensor([P, NT], mybir.dt.float32))
        b_sb = ec(nc.sbuf_tensor([P, 1], mybir.dt.float32))
        ps = [ec(nc.psum_tensor(f"ps{i}", [P, F], mybir.dt.float32)) for i in range(NPSUM)]
        ps_warm = ec(nc.psum_tensor("ps_warm", [P, F], mybir.dt.float32))

        ca_v = cw_sb[:, :, OFF_CA : OFF_CA + F]
        cb_v = cw_sb[:, :, OFF_CB : OFF_CB + CB_N]
        wa_v = cw_sb[:, :, OFF_WA : OFF_WA + F]
        wb_v = cw_sb[:, :, OFF_WB : OFF_WB + F]

        s_cw = ec(nc.semaphore(name="s_cw"))
        s_lin = ec(nc.semaphore(name="s_lin"))
        s_b = ec(nc.semaphore(name="s_b"))
        s_xt = [ec(nc.semaphore(name=f"s_xt{i}")) for i in range(len(XT_G))]
        s_xe = [ec(nc.semaphore(name=f"s_xe{i}")) for i in range(len(XE_G))]
        s_mm = ec(nc.semaphore(name="s_mm"))    # +1 per tile (tensor)
        s_sq = ec(nc.semaphore(name="s_sq"))    # +1 per eigen tile (scalar)
        s_d1 = ec(nc.semaphore(name="s_d1"))    # +1 per direct tile (DVE)
        s_mg = ec(nc.semaphore(name="s_mg"))    # +1 per merged 4-tile group (DVE)
        s_act = ec(nc.semaphore(name="s_act"))  # +1 per sigmoid group (scalar)
        s_out = ec(nc.semaphore(name="s_out"))

        block = ec(nc.Block())

        @block.sync
        def _(sync):
            sync.dma_start(out=cw_sb[:], in_=cw_d).then_inc(s_cw, 16)
            # interleave xt groups with direct-X groups by first-use time
            xe_left = list(range(len(XE_G)))
            lin_sent = False
            for gi, (t0, n) in enumerate(XT_G):
                sync.dma_start(
                    out=xtbuf[:, t0 : t0 + n], in_=xt_d[:, t0 : t0 + n]
                ).then_inc(s_xt[gi], 16)
                while xe_left and T_D[XE_G[xe_left[0]][0]] <= t0 + n:
                    gj = xe_left.pop(0)
                    u0, un = XE_G[gj]
                    sync.dma_start(
                        out=xbuf[:, u0 : u0 + un], in_=xn_d[:, u0 : u0 + un]
                    ).then_inc(s_xe[gj], 16)
                if not lin_sent and gi >= 2:
                    sync.dma_start(out=lin_sb[:], in_=lin_d).then_inc(s_lin, 16)
                    lin_sent = True
            for gj in xe_left:
                u0, un = XE_G[gj]
                sync.dma_start(
                    out=xbuf[:, u0 : u0 + un], in_=xn_d[:, u0 : u0 + un]
                ).then_inc(s_xe[gj], 16)
            # outputs: one store per sigmoid group (4 tiles)
            for yo in range(NT // 4):
                sync.wait_ge(s_act, yo + 1)
                sync.dma_start(
                    out=y[:, 4 * yo : 4 * yo + 4], in_=out_sb[:, 4 * yo : 4 * yo + 4]
                ).then_inc(s_out, 16)
            sync.wait_ge(s_out, 16 * (NT // 4))

        @block.scalar
        def _(scalar):
            scalar.dma_start(out=b_sb[:], in_=bias.to_broadcast([P, 1])).then_inc(s_b, 16)
            scalar.wait_ge(s_b, 16)
            # dummy sigmoid: pull the ~1.3us ACT_TABLE_LOAD (set contains
            # both sigmoid and square) off the critical path
            nc.scalar.activation(
                out=out_sb[:, 0:1],
                in_=b_sb[:],
                func=mybir.ActivationFunctionType.Sigmoid,
                bias=b_sb[:],
                scale=1.0,
            )

            def sigmoid(m):
                scalar.wait_ge(s_mg, m + 1)
                nc.scalar.activation(
                    out=out_sb[:, 4 * m : 4 * m + 4],
                    in_=accm[:, 4 * m : 4 * m + 4],
                    func=mybir.ActivationFunctionType.Sigmoid,
                    bias=b_sb[:],
                    scale=float(2.0 ** (-sc_c)),
                ).then_inc(s_act, 1)

            next_m = 0
            for j, t in enumerate(T_E):
                scalar.wait_ge(s_mm, t + 1)
                bank = ps[t % NPSUM]
                nc.scalar.activation(
                    out=dump_s[:, 0:p_pos],
                    in_=bank[:, 0:p_pos],
                    func=mybir.ActivationFunctionType.Square,
                    accum_out=acc_p[:, j : j + 1],
                )
                nc.scalar.activation(
                    out=dump_s[:, p_pos:],
                    in_=bank[:, p_pos:],
                    func=mybir.ActivationFunctionType.Square,
                    accum_out=acc_n[:, j : j + 1],
                ).then_inc(s_sq, 1)
                # sigmoids lag: emit group m once the square stream reached
                # tile 4m+12 (s_mg(m) is produced around tile 4m+8 on DVE)
                while next_m < NT // 4 and t >= 4 * next_m + 12:
                    sigmoid(next_m)
                    next_m += 1
            while next_m < NT // 4:
                sigmoid(next_m)
                next_m += 1

        @block.tensor
        def _(tensor):
            for _w in range(NWARM):
                nc.tensor.matmul(
                    ps_warm[:],
                    xtbuf[:, 0, 0, :, :],
                    ca_v,
                    start=True,
                    stop=True,
                    perf_mode=DR,
                    skip_group_check=True,
                )
            tensor.wait_ge(s_cw, 16)
            for t in range(NT):
                gi = xt_of[t]
                if t == XT_G[gi][0]:
                    tensor.wait_ge(s_xt[gi], 16)
                if t >= NPSUM:
                    # psum bank reuse: tile t-NPSUM's consumer must be done
                    tp = t - NPSUM
                    if IS_E[tp]:
                        tensor.wait_ge(s_sq, ORD_E[tp] + 1)
                    else:
                        tensor.wait_ge(s_d1, ORD_D[tp] + 1)
                bank = ps[t % NPSUM]
                if IS_E[t]:
                    nc.tensor.matmul(
                        bank[:],
                        xtbuf[:, t, 0, :, :],
                        wa_v,
                        start=True,
                        stop=False,
                        perf_mode=DR,
                        skip_group_check=True,
                    )
                    mm = nc.tensor.matmul(
                        bank[:],
                        xtbuf[:, t, 1, :, :],
                        wb_v,
                        start=False,
                        stop=True,
                        perf_mode=DR,
                        skip_group_check=True,
                    )
                else:
                    nc.tensor.matmul(
                        bank[:],
                        xtbuf[:, t, 0, :, :],
                        ca_v,
                        start=True,
                        stop=False,
                        perf_mode=DR,
                        skip_group_check=True,
                    )
                    mm = nc.tensor.matmul(
                        bank[:, CB_J0:],
                        xtbuf[:, t, 1, :, :],
                        cb_v,
                        start=False,
                        stop=True,
                        perf_mode=DR,
                        skip_group_check=True,
                    )
                mm.then_inc(s_mm, 1)

        @block.vector
        def _(vector):
            vector.wait_ge(s_lin, 16)

            # Merge schedule: phase1(m) (eigen acc_p-acc_n -> tmp2, cross-
            # engine reads only) goes after the first direct-tile STT at
            # t >= 4m+6; phase2(m) (accm <- kappa*tmp2+lin and acc1+lin)
            # goes one direct tile later (>=3 DVE ops after phase1 and far
            # from the acc1 writes it reads).
            def phase1(m):
                les = [ORD_E[t] for t in range(4 * m, 4 * m + 4) if IS_E[t]]
                vector.wait_ge(s_sq, les[-1] + 1)
                assert les == list(range(les[0], les[0] + len(les)))
                nc.vector.scalar_tensor_tensor(
                    out=tmp2[:, m % 2, 0 : len(les)],
                    in0=acc_p[:, les[0] : les[0] + len(les)],
                    scalar=1.0,
                    in1=acc_n[:, les[0] : les[0] + len(les)],
                    op0=mybir.AluOpType.mult,
                    op1=mybir.AluOpType.subtract,
                )

            def phase2(m):
                ets = [t for t in range(4 * m, 4 * m + 4) if IS_E[t]]
                dts = [t for t in range(4 * m, 4 * m + 4) if not IS_E[t]]
                # eigen columns (1 or 2, stride-2 when 2)
                step = ets[1] - ets[0] if len(ets) == 2 else 1
                nc.vector.scalar_tensor_tensor(
                    out=accm[:, ets[0] : ets[-1] + 1 : step],
                    in0=tmp2[:, m % 2, 0 : len(ets)],
                    scalar=kappa,
                    in1=lin_sb[:, ets[0] : ets[-1] + 1 : step],
                    op0=mybir.AluOpType.mult,
                    op1=mybir.AluOpType.add,
                )
                # direct columns as uniform-stride runs
                runs = []
                for t in dts:
                    if runs and len(runs[-1]) == 1:
                        runs[-1].append(t)
                    elif runs and len(runs[-1]) > 1 and t - runs[-1][-1] == runs[-1][1] - runs[-1][0]:
                        runs[-1].append(t)
                    else:
                        runs.append([t])
                last = None
                for r in runs:
                    st = r[1] - r[0] if len(r) > 1 else 1
                    last = nc.vector.scalar_tensor_tensor(
                        out=accm[:, r[0] : r[-1] + 1 : st],
                        in0=acc1[:, r[0] : r[-1] + 1 : st],
                        scalar=1.0,
                        in1=lin_sb[:, r[0] : r[-1] + 1 : st],
                        op0=mybir.AluOpType.mult,
                        op1=mybir.AluOpType.add,
                    )
                last.then_inc(s_mg, 1)

            p1 = 0   # next group to phase1
            p2 = 0   # next group to phase2
            for e, t in enumerate(T_D):
                gi = xe_of[e]
                if e == XE_G[gi][0]:
                    vector.wait_ge(s_xe[gi], 16)
                vector.wait_ge(s_mm, t + 1)
                nc.vector.scalar_tensor_tensor(
                    out=dump[:],
                    in0=ps[t % NPSUM][:],
                    scalar=0.0,
                    in1=xbuf[:, e, :],
                    op0=mybir.AluOpType.add,
                    op1=mybir.AluOpType.mult,
                    accum_out=acc1[:, t : t + 1],
                ).then_inc(s_d1, 1)
                if p2 < p1 and t >= 4 * p2 + 8:
                    phase2(p2)
                    p2 += 1
                if p1 < NT // 4 and t >= 4 * p1 + 6:
                    phase1(p1)
                    p1 += 1
            while p1 < NT // 4:
                phase1(p1)
                p1 += 1
                if p2 < p1 - 1:
                    phase2(p2)
                    p2 += 1
            while p2 < NT // 4:
                phase2(p2)
                p2 += 1

    return nc


def _host_prep(X, w1, b, v, feature2field):
    """Returns (sc_c, sc_w, p_pos, per-core input maps)."""
    X = np.asarray(X, dtype=np.float32)
    w1 = np.asarray(w1, dtype=np.float32)
    b = np.asarray(b, dtype=np.float32)
    v = np.asarray(v, dtype=np.float32)
    f2f = np.asarray(feature2field, dtype=np.int32)

    # Pair-coefficient matrix: C[i,j] = sum_k v[i, f2f[j], k] * v[j, f2f[i], k]
    A = v[:, f2f, :]                      # [n, n, k]
    C = (A * A.transpose(1, 0, 2)).sum(axis=2)
    Cm = np.triu(C, 1)

    # Eigen form: S = Cm + Cm^T = Q diag(lam) Q^T;  inter = sum lam/2 * z^2
    S = Cm + Cm.T
    lam, Q = np.linalg.eigh(S)
    order = np.argsort(-lam)              # positive lambdas first
    lam = lam[order]
    Q = Q[:, order]
    p_pos = int((lam > 0).sum())
    W = Q * np.sqrt(np.abs(lam) / 2.0)[None, :]     # [F, F]

    def scale_pow(m):
        return int(np.floor(np.log2(160.0 / max(float(m), 1e-30))))

    sc_c = scale_pow(np.abs(Cm).max())
    sc_w = scale_pow(np.abs(W).max())
    C8 = (Cm * (2.0 ** sc_c)).astype(FP8)
    W8 = (W * (2.0 ** sc_w)).astype(FP8)

    C8r = C8.reshape(4, P, F)
    W8r = W8.reshape(4, P, F)
    # packed rhs: [ca | cb | wa | wb] along the last axis
    cw = np.concatenate(
        [
            np.stack([C8r[KM[0][0]], C8r[KM[0][1]]], axis=1),
            np.stack([C8r[KM[1][0], :, CB_J0:], C8r[KM[1][1], :, CB_J0:]], axis=1),
            np.stack([W8r[KM[0][0]], W8r[KM[0][1]]], axis=1),
            np.stack([W8r[KM[1][0]], W8r[KM[1][1]]], axis=1),
        ],
        axis=2,
    )
    cw = np.ascontiguousarray(cw)

    X8 = X.astype(FP8)
    Xb = X.astype(BF16)
    linv = (X @ w1[:, 0]) * (2.0 ** sc_c)           # [B] fp32, pre-scaled

    in_maps = []
    for c in range(NCORES):
        X8c = X8[c * BSH : (c + 1) * BSH]
        # xt[p, t, ch, kt, b] = X8c[t*P + b, KM[ch][kt]*P + p]
        x4 = X8c.reshape(NT, P, 4, P)               # [t, b, ktile, p]
        xt = np.ascontiguousarray(
            np.stack(
                [
                    np.stack([x4[:, :, KM[0][0]], x4[:, :, KM[0][1]]], axis=0),
                    np.stack([x4[:, :, KM[1][0]], x4[:, :, KM[1][1]]], axis=0),
                ],
                axis=0,
            ).transpose(4, 2, 0, 1, 3)              # [p, t, ch, kt, b]
        )
        # natural-layout bf16 X for DIRECT tiles only
        Xbc = Xb[c * BSH : (c + 1) * BSH].reshape(NT, P, F)
        xn = np.ascontiguousarray(Xbc[T_D].transpose(1, 0, 2))    # [p, e, f]
        lc = linv[c * BSH : (c + 1) * BSH].reshape(NT, P)
        lin = np.ascontiguousarray(lc.T)                          # [p, t]
        in_maps.append({"xn": xn, "xt": xt, "cw": cw, "lin": lin, "bias": b})
    return sc_c, sc_w, p_pos, in_maps


def _run(prep, trace=False):
    from concourse.bass_utils import run_bass_kernel_spmd

    sc_c, sc_w, p_pos, in_maps = prep
    nc = _build_bass(sc_c, sc_w, p_pos)
    res = run_bass_kernel_spmd(nc, in_maps, core_ids=list(range(NCORES)), trace=trace)
    out = np.concatenate([r["y"].reshape(P, NT).T.reshape(-1) for r in res.results])
    return out, res


def kernel(X, w1, b, v, feature2field):
    prep = _host_prep(X, w1, b, v, feature2field)
    out, _ = _run(prep, trace=False)
    return out.astype(np.float32)


if __name__ == "__main__":
    pass
